# revision 27
# baseline (speedup 1.0000x reference)
"""Causal multi-head attention block, sharded over 8 TRN2 NeuronCores.

Sharding: core c handles batch b = c//2 and head-group g = c%2 (8 of 16 heads).
Each core computes QKV projections, causal flash-style attention, and a
partial output projection for its head group; the host sums the two
head-group partials per batch (partial-sum unshard) and adds b_O.

On-device layouts (per core, S=2048, M=1024, H8=8 heads, Dh=64):
  xall    [128, 16K]  x[b]^T in (superblock, k-chunk) tiles        bf16
  QT/KT   4 pair-tiles [128, 2048]: partition = (head-in-pair, d)  bf16
  Vones   16 s-tiles [128, 520]: row=key pos, col=65*h+d, d=64 → 1 bf16
  zT_all  4 pair-tiles [128, 2048] (normalized z^T)                bf16
  out_t   [1024, 2048]  partial (out proj)^T, host sums + transposes
Scores are computed transposed (S^T[key, query]) so softmax denominators
come from an extra all-ones column in V (matmul partition reduction), and
no on-chip transposes are needed anywhere. The per-block score matmuls
for the two heads of a pair run CONCURRENTLY in the PE array (64-row
tiles at row groups 0/64).

Perf structure (v3 — superblock-major):
 - Attention runs j-major (for j: for p) so each superblock's output
   projection becomes filler for the NEXT superblock's attention; only
   the pair-3 rank-1 update of the last superblock trails the final
   attention.
 - ~20 warm-up matmuls heat the PE HAM clock gate and bridge the startup
   DMA ramp. Input DMAs are coalesced into 0.5-1MB transfers (128KB
   chunks pay a ~2.8us latency floor each) and spread across the
   sync/scalar/gpsimd queues by first-need time; out-proj DMAs round-
   robin the three queues so the write path never backs up the PE.
 - A filler queue (v/qk/out/chain matmul generators) drips ~deficit-
   sized chunks into the ACT-bound attention stream; drain_until forces
   any remainder right before its consumer, and z_mms lazily force-
   drains v(kb) to keep the PE FIFO deadlock-free. Dummy matmuls keep
   the HAM activity monitor from re-throttling when the queue runs dry.
 - Softmax denominators for a superblock collect into one [97, 1024]
   tile (pair p at partition 32p, head at free offset 512*h2 — engine
   APs need contiguous 32-aligned partitions). Reciprocals run as
   Ln+Exp(-x) on ACT (partition-parallel, same table set as the
   attention exps): pairs 0-2 right after att(2,j) so their chains drip
   under att(3,j); pair 3 at the boundary. Two per-pair bc matmuls
   broadcast 1/den into disjoint 64-partition col groups of one PSUM
   bank (concurrent), then one DVE multiply normalizes the whole pair.
 - V-bias is pre-broadcast by the host to 128 partitions, so the v copy
   becomes a DVE add (no bias matmuls).
 - PSUM: scores 2x [128,1024] + zpool 2x + fpool 2x [128,512]: 8 banks.
 - Last superblock: out-proj pairs 0-2 accumulate into bf16 partials as
   filler under att(3,3); the tail reads pair 3's den rows straight from
   the live z PSUM banks (no copy), then 8 rank-update matmuls + adds +
   DMA finish the kernel.
"""

import sys

if "/opt/trn_rl_repo" not in sys.path:
    sys.path.insert(0, "/opt/trn_rl_repo")

import numpy as np
import ml_dtypes

import concourse.bass as bass
import concourse.mybir as mybir
from concourse import tile

BF16 = mybir.dt.bfloat16
F32 = mybir.dt.float32

B, S, M, H, DH = 4, 2048, 1024, 16, 64
H8 = 8          # heads per core
NP = 4          # head pairs per core
SB = 512        # query superblock
KB = 128        # key block
NSB = S // SB   # 4
NKB = S // KB   # 16
MK = M // 128   # 8 contraction chunks
ATTN_SCALE = 1.0 / np.sqrt(DH)
N_WARMUP = 20   # PE warm-up matmuls; also bridges the startup DMA ramp

# ---------------------------------------------------------------------------
# Patch: this walrus build rejects >1 sync-wait per engine instruction.
# Post-pass: for any non-DMA instruction with N>1 waits, insert N-1
# single-wait NoOps on the same engine immediately before it.
MAX_ENGINE_WAITS = 1


def split_multi_waits(nc: bass.Bass):
    n_split = 0
    for f in nc.m.functions:
        for blk in f.blocks:
            new_list = []
            for inst in blk.instructions:
                si = getattr(inst, "sync_info", None)
                waits = list(si.on_wait) if si is not None else []
                if len(waits) > MAX_ENGINE_WAITS:
                    extra = waits[: -MAX_ENGINE_WAITS]
                    keep = waits[-MAX_ENGINE_WAITS:]
                    for i in range(0, len(extra), MAX_ENGINE_WAITS):
                        nop = mybir.InstNoOp(
                            name=f"I-wsplit-{nc.next_id()}", ins=[], outs=[]
                        )
                        nop.engine = inst.engine
                        nop.sync_info = mybir.SyncInfo(
                            on_wait=extra[i : i + MAX_ENGINE_WAITS], on_update=[]
                        )
                        new_list.append(nop)
                    inst.sync_info = mybir.SyncInfo(
                        on_wait=keep, on_update=list(si.on_update)
                    )
                    n_split += 1
                new_list.append(inst)
            blk.instructions = new_list
    return n_split


def build_nc() -> bass.Bass:
    nc = bass.Bass()

    x_t = nc.declare_dram_parameter("x_t", [M, S], BF16, isOutput=False)
    # w_q/w_k pre-tiled p-major: [128, (p, k, 128)] so each pair's weights
    # are one contiguous 0.5MB slab (loaded per pair, just in time).
    w_q = nc.declare_dram_parameter("w_q", [128, NP * MK * 128], BF16, isOutput=False)
    w_k = nc.declare_dram_parameter("w_k", [128, NP * MK * 128], BF16, isOutput=False)
    w_v = nc.declare_dram_parameter("w_v", [128, MK * 512], BF16, isOutput=False)
    w_o = nc.declare_dram_parameter("w_o", [128, NP * MK * 128], BF16, isOutput=False)
    b_q = nc.declare_dram_parameter("b_q", [NP, 128], F32, isOutput=False)
    b_k = nc.declare_dram_parameter("b_k", [NP, 128], F32, isOutput=False)
    # V-bias pre-broadcast by the host across all 128 partitions
    b_v = nc.declare_dram_parameter("b_v", [128, H8 * DH], BF16, isOutput=False)
    out_t = nc.declare_dram_parameter("out_t", [M, S], BF16, isOutput=True)

    with tile.TileContext(nc) as tc:
        with (
            tc.tile_pool(name="persist", bufs=1) as persist,
            tc.tile_pool(name="wstream", bufs=1) as wpool,
        ):
            # --- resident tiles -------------------------------------------
            # x chunks live in one tile, superblock-major, so each
            # superblock loads as one or two large contiguous DMAs (128KB
            # chunk DMAs pay a ~2.8us latency floor each)
            xall = persist.tile([128, NSB * MK * SB], BF16, tag="xall")
            xt = [
                [xall[:, (c * MK + k) * SB : (c * MK + k + 1) * SB] for c in range(NSB)]
                for k in range(MK)
            ]
            qt = [persist.tile([128, S], BF16, tag=f"qt{p}", name=f"qt{p}") for p in range(NP)]
            kt = [persist.tile([128, S], BF16, tag=f"kt{p}", name=f"kt{p}") for p in range(NP)]
            vones = [
                persist.tile([128, H8 * 65], BF16, tag=f"vones{sb}", name=f"vones{sb}")
                for sb in range(NKB)
            ]
            zt = [persist.tile([128, S], BF16, tag=f"zt{p}", name=f"zt{p}") for p in range(NP)]

            wq_all = wpool.tile([128, NP * MK * 128], BF16, tag="wq_all")
            wk_all = wpool.tile([128, NP * MK * 128], BF16, tag="wk_all")
            wv_all = wpool.tile([128, MK * 512], BF16, tag="wv_all")
            wo_all = wpool.tile([128, NP * MK * 128], BF16, tag="wo_all")
            # p-major: pair p, contraction chunk k
            wq = [
                [wq_all[:, (p * MK + k) * 128 : (p * MK + k + 1) * 128] for k in range(MK)]
                for p in range(NP)
            ]
            wk = [
                [wk_all[:, (p * MK + k) * 128 : (p * MK + k + 1) * 128] for k in range(MK)]
                for p in range(NP)
            ]
            wv = [wv_all[:, k * 512 : (k + 1) * 512] for k in range(MK)]
            wo = [
                [wo_all[:, (c * MK + k) * 128 : (c * MK + k + 1) * 128] for k in range(MK)]
                for c in range(NP)
            ]
            bq_t = persist.tile([128, NP], F32, tag="bq")
            bk_t = persist.tile([128, NP], F32, tag="bk")
            bv_t = persist.tile([128, H8 * DH], BF16, tag="bv")
            ones97 = persist.tile([97, 128], BF16, tag="ones97")
            warm_sb = persist.tile([128, 512], BF16, tag="warm_sb")

            # --- memsets (no DMA dependency; run at t=0) ------------------
            nc.gpsimd.memset(warm_sb[:], 1.0)
            nc.gpsimd.memset(ones97[:], 1.0)
            for sb in range(NKB):
                v3 = vones[sb][:].rearrange("p (h e) -> p h e", e=65)
                nc.gpsimd.memset(v3[:, :, 64:65], 1.0)

            # --- PE warm-up: heat the HAM clock gate while DMAs stream ----
            with tc.tile_pool(name="warm_ps", bufs=2, space="PSUM") as warm_pool:
                for _ in range(N_WARMUP):
                    wps = warm_pool.tile([128, 512], F32, tag="warm_ps")
                    nc.tensor.matmul(
                        wps[:], warm_sb[:, 0:128], warm_sb[:], start=True, stop=True
                    )

            # --- DMA program ----------------------------------------------
            # sync: x_t seq-major (compute consumes seq-chunk by chunk),
            # then w_o. scalar: v-path. vector: pairs 0-1 q/k weights.
            # gpsimd: pairs 2-3 q/k weights (j-major needs all four pairs'
            # weights by att(3,0), so they stream in parallel).
            xv = x_t[:].rearrange("(k p) s -> p k s", p=128)

            def dma_x(skb, k0, k1):
                ssl = slice(skb * SB, (skb + 1) * SB)
                dst = xall[
                    :, (skb * MK + k0) * SB : (skb * MK + k1) * SB
                ].rearrange("p (k s) -> p k s", s=SB)
                nc.sync.dma_start(dst, xv[:, k0:k1, ssl])

            dma_x(0, 0, 4)
            dma_x(0, 4, MK)
            # pair-1 q/k weights ride the sync ring right after x sb0 so
            # the gpsimd ring only carries pair 0 (the startup critical path)
            p1sl = slice(1 * MK * 128, 2 * MK * 128)
            nc.sync.dma_start(wq_all[:, p1sl], w_q[:, p1sl])
            nc.sync.dma_start(wk_all[:, p1sl], w_k[:, p1sl])
            for skb in range(1, NSB):
                dma_x(skb, 0, MK)
            nc.sync.dma_start(wo_all[:], w_o[:])
            for k0, k1 in ((0, 4), (4, MK)):
                ksl = slice(k0 * 512, k1 * 512)
                nc.scalar.dma_start(wv_all[:, ksl], w_v[:, ksl])
            nc.scalar.dma_start(bv_t[:], b_v[:])
            for p in range(NP):
                nc.scalar.dma_start(bq_t[:, p], b_q[p])
                nc.scalar.dma_start(bk_t[:, p], b_k[p])
            for p in (0, 2, 3):
                eng = nc.gpsimd if p == 0 else nc.scalar
                psl = slice(p * MK * 128, (p + 1) * MK * 128)
                eng.dma_start(wq_all[:, psl], w_q[:, psl])
                eng.dma_start(wk_all[:, psl], w_k[:, psl])

            # --- main fused phase -----------------------------------------
            # PSUM: sps 2x[128,1024] (4 banks) + zpool 2x (2) + fpool 2x (2)
            with (
                tc.tile_pool(name="fpool", bufs=2, space="PSUM") as fpool,
                tc.tile_pool(name="zpool", bufs=2, space="PSUM") as zpool,
                tc.tile_pool(name="s_ps", bufs=2, space="PSUM") as s_ps,
                tc.tile_pool(name="epool", bufs=10) as epool,
                tc.tile_pool(name="dpool", bufs=2) as dpool,
                tc.tile_pool(name="opool", bufs=6) as opool,
                tc.tile_pool(name="apool", bufs=MK) as apool,
            ):
                # ---- filler stream: projection / out-proj / norm matmuls
                # dripped into the ACT-bound attention loop so the PE never
                # starves while exp runs.
                FEED_NS = 330
                fq = []          # [key, generator]
                done_keys = set()

                def feed(ns):
                    while fq and ns > 0:
                        key, g = fq[0]
                        step = next(g, None)
                        if step is None:
                            done_keys.add(key)
                            fq.pop(0)
                        else:
                            ns -= step
                    return ns

                def drain_until(key):
                    while fq and key not in done_keys:
                        k, g = fq[0]
                        if next(g, None) is None:
                            done_keys.add(k)
                            fq.pop(0)

                def drain_all():
                    while fq:
                        k, g = fq[0]
                        if next(g, None) is None:
                            done_keys.add(k)
                            fq.pop(0)

                def gen_v(kb):
                    skb, r = divmod(kb, 4)
                    ksl = slice(r * KB, (r + 1) * KB)
                    ps_v = fpool.tile([128, 512], F32, tag="fp", name="ps_v")
                    for k in range(MK):
                        nc.tensor.matmul(
                            ps_v[:], xt[k][skb][:, ksl], wv[k],
                            start=(k == 0), stop=(k == MK - 1),
                        )
                        yield 213
                    v3 = vones[kb][:].rearrange("p (h e) -> p h e", e=65)
                    nc.vector.tensor_tensor(
                        v3[:, :, 0:64],
                        ps_v[:].rearrange("p (h e) -> p h e", e=64),
                        bv_t[:].rearrange("p (h e) -> p h e", e=64),
                        op=mybir.AluOpType.add,
                    )

                def gen_qk(p, sb):
                    ssl = slice(sb * SB, (sb + 1) * SB)
                    for w_t, b_t, dst in ((wq, bq_t, qt), (wk, bk_t, kt)):
                        ps = fpool.tile([128, 512], F32, tag="fp", name="ps_qk")
                        for k in range(MK):
                            nc.tensor.matmul(
                                ps[:], w_t[p][k], xt[k][sb][:],
                                start=(k == 0), stop=(k == MK - 1),
                            )
                            yield 213
                        nc.vector.tensor_scalar_add(
                            dst[p][:, ssl], ps[:], b_t[:, p : p + 1]
                        )

                # den / rcp staging: per superblock j, [97, 1024] with pair
                # p's denominators at partition 32p, head h2 at free offset
                # 512*h2. Engines need contiguous 32-aligned partition APs,
                # so the batched reciprocal runs over rows 0..96 (the
                # in-between rows are garbage that nothing ever reads) —
                # DVE cost is free-size only. Two bc matmuls per pair
                # broadcast the 1/den rows into disjoint 64-partition col
                # groups of one PSUM bank (they run concurrently), then a
                # single DVE multiply normalizes both heads of zt.
                den_made = {}

                def get_denj(j):
                    if j not in den_made:
                        den_made[j] = (
                            dpool.tile([97, 2 * SB], F32, tag="den", name=f"den{j}", bufs=2),
                            dpool.tile([97, 2 * SB], BF16, tag="rcp", name=f"rcp{j}", bufs=2),
                            dpool.tile([97, 2 * SB], F32, tag="scr", name=f"scr{j}", bufs=2),
                        )
                    return den_made[j]

                def recip_j(j, r0=0, r1=97):
                    # 1/den as exp(-ln(den)) on ACT: partition-parallel
                    # (DVE reciprocal is ~8x slower and single-issue), both
                    # functions live in the same ACT table set as the
                    # attention exps (no table switch), and Exp writes the
                    # bf16 rcp directly. Not in-place so overlapping row
                    # ranges re-run idempotently.
                    den_j, rcp_j, scr_j = get_denj(j)
                    nc.scalar.activation(
                        scr_j[r0:r1, :], den_j[r0:r1, :],
                        mybir.ActivationFunctionType.Ln,
                    )
                    nc.scalar.activation(
                        rcp_j[r0:r1, :], scr_j[r0:r1, :],
                        mybir.ActivationFunctionType.Exp,
                        scale=-1.0,
                    )

                def gen_chain(p, j):
                    qsl = slice(j * SB, (j + 1) * SB)
                    _, rcp_j, _ = get_denj(j)
                    r0 = 32 * p
                    bc = fpool.tile([128, 512], F32, tag="fp", name="bc")
                    # back-to-back so the two col-group tiles run concurrently
                    for h2 in range(2):
                        nc.tensor.matmul(
                            bc[64 * h2 : 64 * h2 + 64, :],
                            ones97[r0 : r0 + 1, 0:64],
                            rcp_j[r0 : r0 + 1, h2 * SB : (h2 + 1) * SB],
                            start=True, stop=True,
                            tile_position=(r0, 64 * h2),
                        )
                    yield 426
                    nc.vector.tensor_tensor(
                        zt[p][:, qsl], zt[p][:, qsl], bc[:],
                        op=mybir.AluOpType.mult,
                    )

                def gen_out(j):
                    qsl = slice(j * SB, (j + 1) * SB)
                    for k in range(MK):
                        ps_o = fpool.tile([128, 512], F32, tag="fp", name="ps_o")
                        for c in range(NP):
                            nc.tensor.matmul(
                                ps_o[:], wo[c][k], zt[c][:, qsl],
                                start=(c == 0), stop=(c == NP - 1),
                            )
                            yield 213
                        ot = opool.tile([128, SB], BF16, tag="ot", name="ot")
                        nc.vector.tensor_copy(ot[:], ps_o[:])
                        deng = (nc.sync, nc.scalar, nc.gpsimd)[k % 3]
                        deng.dma_start(
                            out_t[k * 128 : (k + 1) * 128, qsl], ot[:]
                        )

                # split out-proj for the last superblock: pairs 0-2 become
                # bf16 partials as filler under att(3,3); pair 3's rank
                # update + DMA is the only post-attention tail.
                pA = [None] * MK

                def gen_outA(j):
                    qsl = slice(j * SB, (j + 1) * SB)
                    for k in range(MK):
                        ps_o = fpool.tile([128, 512], F32, tag="fp", name="ps_oA")
                        for c in range(NP - 1):
                            nc.tensor.matmul(
                                ps_o[:], wo[c][k], zt[c][:, qsl],
                                start=(c == 0), stop=(c == NP - 2),
                            )
                            yield 213
                        t = apool.tile([128, SB], BF16, tag="pA", name=f"pA{k}")
                        pA[k] = t
                        nc.vector.tensor_copy(t[:], ps_o[:])

                def gen_outB(j):
                    qsl = slice(j * SB, (j + 1) * SB)
                    for k in range(MK):
                        ps_o = fpool.tile([128, 512], F32, tag="fp", name="ps_oB")
                        nc.tensor.matmul(
                            ps_o[:], wo[NP - 1][k], zt[NP - 1][:, qsl],
                            start=True, stop=True,
                        )
                        yield 213
                        ot = opool.tile([128, SB], BF16, tag="ot", name="ot")
                        nc.vector.tensor_tensor(
                            ot[:], ps_o[:], pA[k][:], op=mybir.AluOpType.add
                        )
                        deng = (nc.sync, nc.scalar, nc.gpsimd)[k % 3]
                        deng.dma_start(
                            out_t[k * 128 : (k + 1) * 128, qsl], ot[:]
                        )

                def enq(key, g):
                    fq.append((key, g))

                last_zps = []

                def attention(p, j, keep_zps=False):
                    qsl = slice(j * SB, (j + 1) * SB)
                    nk = 4 * (j + 1)
                    den_j, _, _ = get_denj(j)
                    zps = [
                        zpool.tile([128, 512], F32, tag="zp", name="zps0"),
                        zpool.tile([128, 512], F32, tag="zp", name="zps1"),
                    ]
                    if keep_zps:
                        last_zps[:] = zps

                    def z_mms(kbz, e_tile):
                        # v(kbz) must be fully EMITTED before a z matmul that
                        # reads vones[kbz] enters the PE FIFO (else the FIFO
                        # head deadlocks on work queued behind it)
                        drain_until(("v", kbz))
                        # crossing blocks: queries < 128i are fully masked
                        i = kbz - 4 * j
                        c0 = 128 * i if i > 0 else 0
                        for h2 in range(2):
                            h = 2 * p + h2
                            nc.tensor.matmul(
                                zps[h2][0:65, c0:],
                                vones[kbz][:, 65 * h : 65 * h + 65],
                                e_tile[:, h2 * SB + c0 : (h2 + 1) * SB],
                                start=(kbz == 0),
                                stop=(kbz == nk - 1),
                            )

                    pend = []
                    for kb in range(nk):
                        i = kb - 4 * j
                        c0 = 128 * i if i > 0 else 0
                        if kb >= 2:
                            # filler BEFORE this block's matmuls: the PE
                            # works these while ACT catches up on exp(kb-2),
                            # whose sps bank the next scores matmul waits on.
                            cols = 2 * (SB - c0)
                            exp_ns = cols / 1.2 + 293
                            pe_ns = 1.5 * cols * 0.4167 + 160
                            if feed(max(0.0, exp_ns - pe_ns)) > 0:
                                # queue dry: tiny dummy matmuls into the unused
                                # partitions of the live zps bank keep the
                                # HAM activity monitor from re-throttling the
                                # PE clock to 1.2 GHz during ACT-paced blocks
                                for _ in range(2 if j == 3 else 1):
                                    nc.tensor.matmul(
                                        zps[kb % 2][96:128, 0:64],
                                        warm_sb[:, 0:32],
                                        warm_sb[:, 0:64],
                                        start=False,
                                        stop=False,
                                        tile_position=(0, 96),
                                        skip_group_check=True,
                                    )
                        ksl = slice(kb * KB, (kb + 1) * KB)
                        sps = s_ps.tile([128, 2 * SB], F32, tag="sps", name="sps")
                        for h2 in range(2):
                            rows = slice(h2 * 64, h2 * 64 + 64)
                            nc.tensor.matmul(
                                sps[:, h2 * SB + c0 : (h2 + 1) * SB],
                                kt[p][rows, ksl],
                                qt[p][rows, j * SB + c0 : (j + 1) * SB],
                            )
                        e = epool.tile([128, 2 * SB], BF16, tag="e", name="e")
                        if c0:
                            e3 = e[:].rearrange("p (h q) -> p h q", q=SB)
                            s3 = sps[:].rearrange("p (h q) -> p h q", q=SB)
                            nc.scalar.activation(
                                e3[:, :, c0:],
                                s3[:, :, c0:],
                                mybir.ActivationFunctionType.Exp,
                                scale=float(ATTN_SCALE),
                            )
                        else:
                            nc.scalar.activation(
                                e[:],
                                sps[:],
                                mybir.ActivationFunctionType.Exp,
                                scale=float(ATTN_SCALE),
                            )
                        if i >= 0:
                            # zero the strictly-upper part of the diagonal
                            # 128-wide stripe
                            e3 = e[:].rearrange("p (h q) -> p h q", q=SB)
                            nc.gpsimd.affine_select(
                                out=e3[:, :, c0 : c0 + 128],
                                in_=e3[:, :, c0 : c0 + 128],
                                compare_op=mybir.AluOpType.is_ge,
                                fill=0.0,
                                base=j * SB - kb * KB + c0,
                                pattern=[[0, 2], [1, 128]],
                                channel_multiplier=-1,
                            )
                        pend.append((kb, e))
                        if len(pend) > 2:
                            z_mms(*pend.pop(0))
                    for it in pend:
                        feed(2 * FEED_NS)
                        z_mms(*it)

                    # stage den rows first so the batched recip and the
                    # next superblock's qk bias-add don't queue behind the
                    # z casts on DVE (gpsimd can't read PSUM). The final
                    # attention keeps its banks alive instead — the tail
                    # recip reads the den rows straight from PSUM.
                    if not keep_zps:
                        for h2 in range(2):
                            nc.vector.tensor_copy(
                                den_j[32 * p : 32 * p + 1, h2 * SB : (h2 + 1) * SB],
                                zps[h2][64:65, :],
                            )
                    for h2 in range(2):
                        rows = slice(h2 * 64, h2 * 64 + 64)
                        nc.vector.tensor_copy(zt[p][rows, qsl], zps[h2][0:64, :])

                # --- pipelined emission (j-major) -------------------------
                # upfront filler queue: v for superblocks 0-1, all qk for
                # j=0/1 (drain_until forces completion where needed)
                for kb in range(4):
                    enq(("v", kb), gen_v(kb))
                for p in range(NP):
                    enq(("qk", p, 0), gen_qk(p, 0))
                for kb in range(4, 8):
                    enq(("v", kb), gen_v(kb))
                for p in range(NP):
                    enq(("qk", p, 1), gen_qk(p, 1))

                for j in range(NSB):
                    for p in range(NP):
                        drain_until(("qk", p, j))
                        attention(p, j, keep_zps=(j == 3 and p == 3))
                        # staggered future-work enqueues
                        if j == 1:
                            if p == 0:
                                enq(("qk", 0, 2), gen_qk(0, 2))
                                enq(("qk", 1, 2), gen_qk(1, 2))
                            elif p == 1:
                                enq(("qk", 2, 2), gen_qk(2, 2))
                                enq(("qk", 3, 2), gen_qk(3, 2))
                            elif p == 2:
                                enq(("v", 8), gen_v(8))
                                enq(("v", 9), gen_v(9))
                            else:
                                enq(("v", 10), gen_v(10))
                                enq(("v", 11), gen_v(11))
                        elif j == 2:
                            if p == 0:
                                enq(("qk", 0, 3), gen_qk(0, 3))
                                enq(("qk", 1, 3), gen_qk(1, 3))
                            elif p == 1:
                                enq(("qk", 2, 3), gen_qk(2, 3))
                                enq(("qk", 3, 3), gen_qk(3, 3))
                            elif p == 2:
                                enq(("v", 12), gen_v(12))
                                enq(("v", 13), gen_v(13))
                            else:
                                enq(("v", 14), gen_v(14))
                                enq(("v", 15), gen_v(15))
                        if p == 2:
                            # pairs 0-2 done for this superblock: their
                            # recips + chains hide under att(3, j)
                            recip_j(j, 0, 65)
                            for c in range(3):
                                enq(("ch", c, j), gen_chain(c, j))
                            if j == 3:
                                enq(("outA",), gen_outA(3))
                    if j < 3:
                        # superblock boundary: only pair 3's reciprocal and
                        # chain remain, then out-proj as next-j filler.
                        # A few dummy matmuls bridge the recip->chain serial
                        # stretch so HAM stays warm.
                        recip_j(j, 64, 97)
                        dcs = zpool.tile([128, 512], F32, tag="zp", name="dcs")
                        for _ in range(3):
                            nc.tensor.matmul(
                                dcs[96:128, 0:64],
                                warm_sb[:, 0:32],
                                warm_sb[:, 0:64],
                                start=True, stop=True,
                                tile_position=(0, 96),
                                skip_group_check=True,
                            )
                        enq(("ch", 3, j), gen_chain(3, j))
                        enq(("out", j), gen_out(j))

                # tail: pair-3 norm + rank update only, pipelined by head
                # (each half's recip/bc starts as soon as its den row lands)
                drain_all()
                den_j, rcp_j, scr_j = get_denj(3)
                qsl = slice(3 * SB, 4 * SB)
                bc = fpool.tile([128, 512], F32, tag="fp", name="bc")
                for h2 in range(2):
                    hsl = slice(h2 * SB, (h2 + 1) * SB)
                    nc.scalar.activation(
                        scr_j[96:97, hsl], last_zps[h2][64:65, :],
                        mybir.ActivationFunctionType.Ln,
                    )
                    nc.scalar.activation(
                        rcp_j[96:97, hsl], scr_j[96:97, hsl],
                        mybir.ActivationFunctionType.Exp,
                        scale=-1.0,
                    )
                    nc.tensor.matmul(
                        bc[64 * h2 : 64 * h2 + 64, :],
                        ones97[96:97, 0:64],
                        rcp_j[96:97, hsl],
                        start=True, stop=True,
                        tile_position=(96, 64 * h2),
                    )
                    nc.vector.tensor_tensor(
                        zt[3][64 * h2 : 64 * h2 + 64, qsl],
                        zt[3][64 * h2 : 64 * h2 + 64, qsl],
                        bc[64 * h2 : 64 * h2 + 64, :],
                        op=mybir.AluOpType.mult,
                    )
                for _ in gen_outB(3):
                    pass

    split_multi_waits(nc)
    return nc


_CACHED = {}


def _get_nc():
    if "nc" not in _CACHED:
        _CACHED["nc"] = build_nc()
    return _CACHED["nc"]


def kernel(
    x,
    pos_embed,
    W_Q,
    b_Q,
    W_K,
    b_K,
    W_V,
    b_V,
    W_O,
    b_O,
    _want_results=False,
    _trace=False,
):
    from concourse.bass_utils import run_bass_kernel_spmd

    bf16 = ml_dtypes.bfloat16
    x = np.asarray(x, np.float32)
    W_Q = np.asarray(W_Q, np.float32)
    b_Q = np.asarray(b_Q, np.float32)
    W_K = np.asarray(W_K, np.float32)
    b_K = np.asarray(b_K, np.float32)
    W_V = np.asarray(W_V, np.float32)
    b_V = np.asarray(b_V, np.float32)
    W_O = np.asarray(W_O, np.float32)
    b_O = np.asarray(b_O, np.float32)

    in_maps = []
    for c in range(8):
        b, g = divmod(c, 2)
        hs = slice(g * H8, (g + 1) * H8)
        # [H8, M, DH] -> [M, H8*DH] with col = 64*h + d, then pre-tiled into
        # the on-chip layout: [128 part, (p, k, 128)] for Q/K (p-major),
        # [128, (k, 512)] for V, [128, (c, k, 128)] for O.
        wq_f = W_Q[hs].transpose(1, 0, 2).reshape(M, H8 * DH)
        wk_f = W_K[hs].transpose(1, 0, 2).reshape(M, H8 * DH)
        wv_f = W_V[hs].transpose(1, 0, 2).reshape(M, H8 * DH)
        wo_f = W_O[hs].reshape(H8 * DH, M)
        # [M=MK*128, NP*128] -> [MK, 128, NP, 128] -> [128, NP, MK, 128]
        wq = np.ascontiguousarray(
            wq_f.reshape(MK, 128, NP, 128).transpose(1, 2, 0, 3).reshape(128, -1)
        )
        wk = np.ascontiguousarray(
            wk_f.reshape(MK, 128, NP, 128).transpose(1, 2, 0, 3).reshape(128, -1)
        )
        wv = np.ascontiguousarray(
            wv_f.reshape(MK, 128, 512).transpose(1, 0, 2).reshape(128, -1)
        )
        # [H8*DH=NP*128, M=MK*128] -> [NP, 128, MK, 128] -> [128, NP, MK, 128]
        wo = np.ascontiguousarray(
            wo_f.reshape(NP, 128, MK, 128).transpose(1, 0, 2, 3).reshape(128, -1)
        )
        bvb = np.broadcast_to(
            b_V[hs].reshape(1, H8 * DH), (128, H8 * DH)
        )
        in_maps.append(
            {
                "x_t": np.ascontiguousarray(x[b].T).astype(bf16),
                "w_q": wq.astype(bf16),
                "w_k": wk.astype(bf16),
                "w_v": wv.astype(bf16),
                "w_o": wo.astype(bf16),
                "b_q": np.ascontiguousarray(b_Q[hs].reshape(NP, 128)),
                "b_k": np.ascontiguousarray(b_K[hs].reshape(NP, 128)),
                "b_v": np.ascontiguousarray(bvb).astype(bf16),
            }
        )

    nc = _get_nc()
    res = run_bass_kernel_spmd(nc, in_maps, list(range(8)), trace=_trace)

    out = np.empty((B, S, M), np.float32)
    for b in range(B):
        p0 = res.results[2 * b]["out_t"].astype(np.float32)
        p1 = res.results[2 * b + 1]["out_t"].astype(np.float32)
        out[b] = (p0 + p1).T + b_O
    if _want_results:
        return out, res
    return out


# revision 29
# speedup vs baseline: 1.0119x; 1.0119x over previous
"""Causal multi-head attention block, sharded over 8 TRN2 NeuronCores.

Sharding: core c handles batch b = c//2 and head-group g = c%2 (8 of 16 heads).
Each core computes QKV projections, causal flash-style attention, and a
partial output projection for its head group; the host sums the two
head-group partials per batch (partial-sum unshard) and adds b_O.

On-device layouts (per core, S=2048, M=1024, H8=8 heads, Dh=64):
  xall    [128, 16K]  x[b]^T in (superblock, k-chunk) tiles        bf16
  QT/KT   4 pair-tiles [128, 2048]: partition = (head-in-pair, d)  bf16
  Vones   16 s-tiles [128, 520]: row=key pos, col=65*h+d, d=64 → 1 bf16
  zT_all  4 pair-tiles [128, 2048] (normalized z^T)                bf16
  out_t   [1024, 2048]  partial (out proj)^T, host sums + transposes
Scores are computed transposed (S^T[key, query]) so softmax denominators
come from an extra all-ones column in V (matmul partition reduction), and
no on-chip transposes are needed anywhere. The per-block score matmuls
for the two heads of a pair run CONCURRENTLY in the PE array (64-row
tiles at row groups 0/64).

Perf structure (v3 — superblock-major):
 - Attention runs j-major (for j: for p) so each superblock's output
   projection becomes filler for the NEXT superblock's attention; only
   the pair-3 rank-1 update of the last superblock trails the final
   attention.
 - ~20 warm-up matmuls heat the PE HAM clock gate and bridge the startup
   DMA ramp. Input DMAs are coalesced into 0.5-1MB transfers (128KB
   chunks pay a ~2.8us latency floor each) and spread across the
   sync/scalar/gpsimd queues by first-need time; out-proj DMAs round-
   robin the three queues so the write path never backs up the PE.
 - A filler queue (v/qk/out/chain matmul generators) drips ~deficit-
   sized chunks into the ACT-bound attention stream; drain_until forces
   any remainder right before its consumer, and z_mms lazily force-
   drains v(kb) to keep the PE FIFO deadlock-free. Dummy matmuls keep
   the HAM activity monitor from re-throttling when the queue runs dry.
 - Softmax denominators for a superblock collect into one [97, 1024]
   tile (pair p at partition 32p, head at free offset 512*h2 — engine
   APs need contiguous 32-aligned partitions). Reciprocals run as
   Ln+Exp(-x) on ACT (partition-parallel, same table set as the
   attention exps): pairs 0-2 right after att(2,j) so their chains drip
   under att(3,j); pair 3 at the boundary. Two per-pair bc matmuls
   broadcast 1/den into disjoint 64-partition col groups of one PSUM
   bank (concurrent), then one DVE multiply normalizes the whole pair.
 - V-bias is pre-broadcast by the host to 128 partitions, so the v copy
   becomes a DVE add (no bias matmuls).
 - PSUM: scores 2x [128,1024] + zpool 2x + fpool 2x [128,512]: 8 banks.
 - Last superblock: out-proj pairs 0-2 accumulate into bf16 partials as
   filler under att(3,3); the tail reads pair 3's den rows straight from
   the live z PSUM banks (no copy), then 8 rank-update matmuls + adds +
   DMA finish the kernel.
"""

import sys

if "/opt/trn_rl_repo" not in sys.path:
    sys.path.insert(0, "/opt/trn_rl_repo")

import numpy as np
import ml_dtypes

import concourse.bass as bass
import concourse.mybir as mybir
from concourse import tile

BF16 = mybir.dt.bfloat16
F32 = mybir.dt.float32

B, S, M, H, DH = 4, 2048, 1024, 16, 64
H8 = 8          # heads per core
NP = 4          # head pairs per core
SB = 512        # query superblock
KB = 128        # key block
NSB = S // SB   # 4
NKB = S // KB   # 16
MK = M // 128   # 8 contraction chunks
ATTN_SCALE = 1.0 / np.sqrt(DH)
N_WARMUP = 20   # PE warm-up matmuls; also bridges the startup DMA ramp

# ---------------------------------------------------------------------------
# Patch: this walrus build rejects >1 sync-wait per engine instruction.
# Post-pass: for any non-DMA instruction with N>1 waits, insert N-1
# single-wait NoOps on the same engine immediately before it.
MAX_ENGINE_WAITS = 1


def split_multi_waits(nc: bass.Bass):
    n_split = 0
    for f in nc.m.functions:
        for blk in f.blocks:
            new_list = []
            for inst in blk.instructions:
                si = getattr(inst, "sync_info", None)
                waits = list(si.on_wait) if si is not None else []
                if len(waits) > MAX_ENGINE_WAITS:
                    extra = waits[: -MAX_ENGINE_WAITS]
                    keep = waits[-MAX_ENGINE_WAITS:]
                    for i in range(0, len(extra), MAX_ENGINE_WAITS):
                        nop = mybir.InstNoOp(
                            name=f"I-wsplit-{nc.next_id()}", ins=[], outs=[]
                        )
                        nop.engine = inst.engine
                        nop.sync_info = mybir.SyncInfo(
                            on_wait=extra[i : i + MAX_ENGINE_WAITS], on_update=[]
                        )
                        new_list.append(nop)
                    inst.sync_info = mybir.SyncInfo(
                        on_wait=keep, on_update=list(si.on_update)
                    )
                    n_split += 1
                new_list.append(inst)
            blk.instructions = new_list
    return n_split


def build_nc() -> bass.Bass:
    nc = bass.Bass()

    x_t = nc.declare_dram_parameter("x_t", [M, S], BF16, isOutput=False)
    # w_q/w_k pre-tiled p-major: [128, (p, k, 128)] so each pair's weights
    # are one contiguous 0.5MB slab (loaded per pair, just in time).
    w_q = nc.declare_dram_parameter("w_q", [128, NP * MK * 128], BF16, isOutput=False)
    w_k = nc.declare_dram_parameter("w_k", [128, NP * MK * 128], BF16, isOutput=False)
    w_v = nc.declare_dram_parameter("w_v", [128, MK * 512], BF16, isOutput=False)
    w_o = nc.declare_dram_parameter("w_o", [128, NP * MK * 128], BF16, isOutput=False)
    b_q = nc.declare_dram_parameter("b_q", [NP, 128], F32, isOutput=False)
    b_k = nc.declare_dram_parameter("b_k", [NP, 128], F32, isOutput=False)
    # V-bias pre-broadcast by the host across all 128 partitions
    b_v = nc.declare_dram_parameter("b_v", [128, H8 * DH], BF16, isOutput=False)
    out_t = nc.declare_dram_parameter("out_t", [M, S], BF16, isOutput=True)

    with tile.TileContext(nc) as tc:
        with (
            tc.tile_pool(name="persist", bufs=1) as persist,
            tc.tile_pool(name="wstream", bufs=1) as wpool,
        ):
            # --- resident tiles -------------------------------------------
            # x chunks live in one tile, superblock-major, so each
            # superblock loads as one or two large contiguous DMAs (128KB
            # chunk DMAs pay a ~2.8us latency floor each)
            xall = persist.tile([128, NSB * MK * SB], BF16, tag="xall")
            xt = [
                [xall[:, (c * MK + k) * SB : (c * MK + k + 1) * SB] for c in range(NSB)]
                for k in range(MK)
            ]
            qt = [persist.tile([128, S], BF16, tag=f"qt{p}", name=f"qt{p}") for p in range(NP)]
            kt = [persist.tile([128, S], BF16, tag=f"kt{p}", name=f"kt{p}") for p in range(NP)]
            vones = [
                persist.tile([128, H8 * 65], BF16, tag=f"vones{sb}", name=f"vones{sb}")
                for sb in range(NKB)
            ]
            zt = [persist.tile([128, S], BF16, tag=f"zt{p}", name=f"zt{p}") for p in range(NP)]

            wq_all = wpool.tile([128, NP * MK * 128], BF16, tag="wq_all")
            wk_all = wpool.tile([128, NP * MK * 128], BF16, tag="wk_all")
            wv_all = wpool.tile([128, MK * 512], BF16, tag="wv_all")
            wo_all = wpool.tile([128, NP * MK * 128], BF16, tag="wo_all")
            # p-major: pair p, contraction chunk k
            wq = [
                [wq_all[:, (p * MK + k) * 128 : (p * MK + k + 1) * 128] for k in range(MK)]
                for p in range(NP)
            ]
            wk = [
                [wk_all[:, (p * MK + k) * 128 : (p * MK + k + 1) * 128] for k in range(MK)]
                for p in range(NP)
            ]
            wv = [wv_all[:, k * 512 : (k + 1) * 512] for k in range(MK)]
            wo = [
                [wo_all[:, (c * MK + k) * 128 : (c * MK + k + 1) * 128] for k in range(MK)]
                for c in range(NP)
            ]
            bq_t = persist.tile([128, NP], F32, tag="bq")
            bk_t = persist.tile([128, NP], F32, tag="bk")
            bv_t = persist.tile([128, H8 * DH], BF16, tag="bv")
            ones97 = persist.tile([97, 128], BF16, tag="ones97")
            warm_sb = persist.tile([128, 512], BF16, tag="warm_sb")

            # --- memsets (no DMA dependency; run at t=0) ------------------
            nc.gpsimd.memset(warm_sb[:], 1.0)
            nc.gpsimd.memset(ones97[:], 1.0)
            for sb in range(NKB):
                v3 = vones[sb][:].rearrange("p (h e) -> p h e", e=65)
                nc.gpsimd.memset(v3[:, :, 64:65], 1.0)

            # --- PE warm-up: heat the HAM clock gate while DMAs stream ----
            with tc.tile_pool(name="warm_ps", bufs=2, space="PSUM") as warm_pool:
                for _ in range(N_WARMUP):
                    wps = warm_pool.tile([128, 512], F32, tag="warm_ps")
                    nc.tensor.matmul(
                        wps[:], warm_sb[:, 0:128], warm_sb[:], start=True, stop=True
                    )

            # --- DMA program ----------------------------------------------
            # sync: x_t seq-major (compute consumes seq-chunk by chunk),
            # then w_o. scalar: v-path. vector: pairs 0-1 q/k weights.
            # gpsimd: pairs 2-3 q/k weights (j-major needs all four pairs'
            # weights by att(3,0), so they stream in parallel).
            xv = x_t[:].rearrange("(k p) s -> p k s", p=128)

            def dma_x(skb, k0, k1):
                ssl = slice(skb * SB, (skb + 1) * SB)
                dst = xall[
                    :, (skb * MK + k0) * SB : (skb * MK + k1) * SB
                ].rearrange("p (k s) -> p k s", s=SB)
                nc.sync.dma_start(dst, xv[:, k0:k1, ssl])

            dma_x(0, 0, 4)
            dma_x(0, 4, MK)
            # pair-1 q/k weights ride the sync ring right after x sb0 so
            # the gpsimd ring only carries pair 0 (the startup critical path)
            p1sl = slice(1 * MK * 128, 2 * MK * 128)
            nc.sync.dma_start(wq_all[:, p1sl], w_q[:, p1sl])
            nc.sync.dma_start(wk_all[:, p1sl], w_k[:, p1sl])
            for skb in range(1, NSB):
                dma_x(skb, 0, MK)
            nc.sync.dma_start(wo_all[:], w_o[:])
            for k0, k1 in ((0, 4), (4, MK)):
                ksl = slice(k0 * 512, k1 * 512)
                nc.scalar.dma_start(wv_all[:, ksl], w_v[:, ksl])
            nc.scalar.dma_start(bv_t[:], b_v[:])
            for p in range(NP):
                nc.scalar.dma_start(bq_t[:, p], b_q[p])
                nc.scalar.dma_start(bk_t[:, p], b_k[p])
            for p in (0, 2, 3):
                eng = nc.gpsimd if p == 0 else nc.scalar
                psl = slice(p * MK * 128, (p + 1) * MK * 128)
                eng.dma_start(wq_all[:, psl], w_q[:, psl])
                eng.dma_start(wk_all[:, psl], w_k[:, psl])

            # --- main fused phase -----------------------------------------
            # PSUM: sps 2x[128,1024] (4 banks) + zpool 2x (2) + fpool 2x (2)
            with (
                tc.tile_pool(name="fpool", bufs=2, space="PSUM") as fpool,
                tc.tile_pool(name="zpool", bufs=2, space="PSUM") as zpool,
                tc.tile_pool(name="s_ps", bufs=2, space="PSUM") as s_ps,
                tc.tile_pool(name="epool", bufs=10) as epool,
                tc.tile_pool(name="dpool", bufs=2) as dpool,
                tc.tile_pool(name="opool", bufs=6) as opool,
                tc.tile_pool(name="apool", bufs=MK) as apool,
            ):
                # ---- filler stream: projection / out-proj / norm matmuls
                # dripped into the ACT-bound attention loop so the PE never
                # starves while exp runs.
                FEED_NS = 330
                fq = []          # [key, generator]
                done_keys = set()

                def feed(ns):
                    while fq and ns > 0:
                        key, g = fq[0]
                        step = next(g, None)
                        if step is None:
                            done_keys.add(key)
                            fq.pop(0)
                        else:
                            ns -= step
                    return ns

                def drain_until(key):
                    while fq and key not in done_keys:
                        k, g = fq[0]
                        if next(g, None) is None:
                            done_keys.add(k)
                            fq.pop(0)

                def drain_all():
                    while fq:
                        k, g = fq[0]
                        if next(g, None) is None:
                            done_keys.add(k)
                            fq.pop(0)

                def gen_v(kb):
                    skb, r = divmod(kb, 4)
                    ksl = slice(r * KB, (r + 1) * KB)
                    ps_v = fpool.tile([128, 512], F32, tag="fp", name="ps_v")
                    for k in range(MK):
                        nc.tensor.matmul(
                            ps_v[:], xt[k][skb][:, ksl], wv[k],
                            start=(k == 0), stop=(k == MK - 1),
                        )
                        yield 213
                    v3 = vones[kb][:].rearrange("p (h e) -> p h e", e=65)
                    nc.vector.tensor_tensor(
                        v3[:, :, 0:64],
                        ps_v[:].rearrange("p (h e) -> p h e", e=64),
                        bv_t[:].rearrange("p (h e) -> p h e", e=64),
                        op=mybir.AluOpType.add,
                    )

                def gen_qk(p, sb):
                    ssl = slice(sb * SB, (sb + 1) * SB)
                    for w_t, b_t, dst in ((wq, bq_t, qt), (wk, bk_t, kt)):
                        ps = fpool.tile([128, 512], F32, tag="fp", name="ps_qk")
                        for k in range(MK):
                            nc.tensor.matmul(
                                ps[:], w_t[p][k], xt[k][sb][:],
                                start=(k == 0), stop=(k == MK - 1),
                            )
                            yield 213
                        nc.vector.tensor_scalar_add(
                            dst[p][:, ssl], ps[:], b_t[:, p : p + 1]
                        )

                # den / rcp staging: per superblock j, [97, 1024] with pair
                # p's denominators at partition 32p, head h2 at free offset
                # 512*h2. Engines need contiguous 32-aligned partition APs,
                # so the batched reciprocal runs over rows 0..96 (the
                # in-between rows are garbage that nothing ever reads) —
                # DVE cost is free-size only. Two bc matmuls per pair
                # broadcast the 1/den rows into disjoint 64-partition col
                # groups of one PSUM bank (they run concurrently), then a
                # single DVE multiply normalizes both heads of zt.
                den_made = {}

                def get_denj(j):
                    if j not in den_made:
                        den_made[j] = (
                            dpool.tile([97, 2 * SB], F32, tag="den", name=f"den{j}", bufs=2),
                            dpool.tile([97, 2 * SB], BF16, tag="rcp", name=f"rcp{j}", bufs=2),
                            dpool.tile([97, 2 * SB], F32, tag="scr", name=f"scr{j}", bufs=2),
                        )
                    return den_made[j]

                def recip_j(j, r0=0, r1=97):
                    # 1/den as exp(-ln(den)) on ACT: partition-parallel
                    # (DVE reciprocal is ~8x slower and single-issue), both
                    # functions live in the same ACT table set as the
                    # attention exps (no table switch), and Exp writes the
                    # bf16 rcp directly. Not in-place so overlapping row
                    # ranges re-run idempotently.
                    den_j, rcp_j, scr_j = get_denj(j)
                    nc.scalar.activation(
                        scr_j[r0:r1, :], den_j[r0:r1, :],
                        mybir.ActivationFunctionType.Ln,
                    )
                    nc.scalar.activation(
                        rcp_j[r0:r1, :], scr_j[r0:r1, :],
                        mybir.ActivationFunctionType.Exp,
                        scale=-1.0,
                    )

                def gen_chain(p, j):
                    qsl = slice(j * SB, (j + 1) * SB)
                    _, rcp_j, _ = get_denj(j)
                    r0 = 32 * p
                    bc = fpool.tile([128, 512], F32, tag="fp", name="bc")
                    # back-to-back so the two col-group tiles run concurrently
                    for h2 in range(2):
                        nc.tensor.matmul(
                            bc[64 * h2 : 64 * h2 + 64, :],
                            ones97[r0 : r0 + 1, 0:64],
                            rcp_j[r0 : r0 + 1, h2 * SB : (h2 + 1) * SB],
                            start=True, stop=True,
                            tile_position=(r0, 64 * h2),
                        )
                    yield 426
                    nc.vector.tensor_tensor(
                        zt[p][:, qsl], zt[p][:, qsl], bc[:],
                        op=mybir.AluOpType.mult,
                    )

                def gen_out(j):
                    qsl = slice(j * SB, (j + 1) * SB)
                    for k in range(MK):
                        ps_o = fpool.tile([128, 512], F32, tag="fp", name="ps_o")
                        for c in range(NP):
                            nc.tensor.matmul(
                                ps_o[:], wo[c][k], zt[c][:, qsl],
                                start=(c == 0), stop=(c == NP - 1),
                            )
                            yield 213
                        ot = opool.tile([128, SB], BF16, tag="ot", name="ot")
                        nc.vector.tensor_copy(ot[:], ps_o[:])
                        deng = (nc.sync, nc.scalar, nc.gpsimd)[k % 3]
                        deng.dma_start(
                            out_t[k * 128 : (k + 1) * 128, qsl], ot[:]
                        )

                # split out-proj for the last superblock: pairs 0-2 become
                # bf16 partials as filler under att(3,3); pair 3's rank
                # update + DMA is the only post-attention tail.
                pA = [None] * MK

                def gen_outA(j):
                    qsl = slice(j * SB, (j + 1) * SB)
                    for k in range(MK):
                        ps_o = fpool.tile([128, 512], F32, tag="fp", name="ps_oA")
                        for c in range(NP - 1):
                            nc.tensor.matmul(
                                ps_o[:], wo[c][k], zt[c][:, qsl],
                                start=(c == 0), stop=(c == NP - 2),
                            )
                            yield 213
                        t = apool.tile([128, SB], BF16, tag="pA", name=f"pA{k}")
                        pA[k] = t
                        nc.vector.tensor_copy(t[:], ps_o[:])

                def gen_outB(j):
                    qsl = slice(j * SB, (j + 1) * SB)
                    for k in range(MK):
                        ps_o = fpool.tile([128, 512], F32, tag="fp", name="ps_oB")
                        nc.tensor.matmul(
                            ps_o[:], wo[NP - 1][k], zt[NP - 1][:, qsl],
                            start=True, stop=True,
                        )
                        yield 213
                        ot = opool.tile([128, SB], BF16, tag="ot", name="ot")
                        nc.vector.tensor_tensor(
                            ot[:], ps_o[:], pA[k][:], op=mybir.AluOpType.add
                        )
                        deng = (nc.sync, nc.scalar, nc.gpsimd)[k % 3]
                        deng.dma_start(
                            out_t[k * 128 : (k + 1) * 128, qsl], ot[:]
                        )

                def enq(key, g):
                    fq.append((key, g))

                last_zps = []

                def attention(p, j, keep_zps=False):
                    qsl = slice(j * SB, (j + 1) * SB)
                    nk = 4 * (j + 1)
                    den_j, _, _ = get_denj(j)
                    zps = [
                        zpool.tile([128, 512], F32, tag="zp", name="zps0"),
                        zpool.tile([128, 512], F32, tag="zp", name="zps1"),
                    ]
                    if keep_zps:
                        last_zps[:] = zps

                    def z_mms(kbz, e_tile):
                        # v(kbz) must be fully EMITTED before a z matmul that
                        # reads vones[kbz] enters the PE FIFO (else the FIFO
                        # head deadlocks on work queued behind it)
                        drain_until(("v", kbz))
                        # crossing blocks: queries < 128i are fully masked
                        i = kbz - 4 * j
                        c0 = 128 * i if i > 0 else 0
                        for h2 in range(2):
                            h = 2 * p + h2
                            nc.tensor.matmul(
                                zps[h2][0:65, c0:],
                                vones[kbz][:, 65 * h : 65 * h + 65],
                                e_tile[:, h2 * SB + c0 : (h2 + 1) * SB],
                                start=(kbz == 0),
                                stop=(kbz == nk - 1),
                            )

                    pend = []
                    for kb in range(nk):
                        i = kb - 4 * j
                        c0 = 128 * i if i > 0 else 0
                        if kb >= 2:
                            # filler BEFORE this block's matmuls: the PE
                            # works these while ACT catches up on exp(kb-2),
                            # whose sps bank the next scores matmul waits on.
                            cols = 2 * (SB - c0)
                            exp_ns = cols / 1.2 + 293
                            pe_ns = 1.5 * cols * 0.4167 + 160
                            if feed(max(0.0, exp_ns - pe_ns)) > 0:
                                # queue dry: tiny dummy matmuls into the unused
                                # partitions of the live zps bank keep the
                                # HAM activity monitor from re-throttling the
                                # PE clock to 1.2 GHz during ACT-paced blocks
                                for _ in range(2 if j == 3 else 1):
                                    nc.tensor.matmul(
                                        zps[kb % 2][96:128, 0:64],
                                        warm_sb[:, 0:32],
                                        warm_sb[:, 0:64],
                                        start=False,
                                        stop=False,
                                        tile_position=(0, 96),
                                        skip_group_check=True,
                                    )
                        ksl = slice(kb * KB, (kb + 1) * KB)
                        sps = s_ps.tile([128, 2 * SB], F32, tag="sps", name="sps")
                        for h2 in range(2):
                            rows = slice(h2 * 64, h2 * 64 + 64)
                            nc.tensor.matmul(
                                sps[:, h2 * SB + c0 : (h2 + 1) * SB],
                                kt[p][rows, ksl],
                                qt[p][rows, j * SB + c0 : (j + 1) * SB],
                            )
                        e = epool.tile([128, 2 * SB], BF16, tag="e", name="e")
                        if c0:
                            e3 = e[:].rearrange("p (h q) -> p h q", q=SB)
                            s3 = sps[:].rearrange("p (h q) -> p h q", q=SB)
                            nc.scalar.activation(
                                e3[:, :, c0:],
                                s3[:, :, c0:],
                                mybir.ActivationFunctionType.Exp,
                                scale=float(ATTN_SCALE),
                            )
                        else:
                            nc.scalar.activation(
                                e[:],
                                sps[:],
                                mybir.ActivationFunctionType.Exp,
                                scale=float(ATTN_SCALE),
                            )
                        if i >= 0:
                            # zero the strictly-upper part of the diagonal
                            # 128-wide stripe
                            e3 = e[:].rearrange("p (h q) -> p h q", q=SB)
                            nc.gpsimd.affine_select(
                                out=e3[:, :, c0 : c0 + 128],
                                in_=e3[:, :, c0 : c0 + 128],
                                compare_op=mybir.AluOpType.is_ge,
                                fill=0.0,
                                base=j * SB - kb * KB + c0,
                                pattern=[[0, 2], [1, 128]],
                                channel_multiplier=-1,
                            )
                        pend.append((kb, e))
                        if len(pend) > 2:
                            z_mms(*pend.pop(0))
                    for it in pend:
                        feed(2 * FEED_NS)
                        z_mms(*it)

                    # stage den rows first so the batched recip and the
                    # next superblock's qk bias-add don't queue behind the
                    # z casts on DVE (gpsimd can't read PSUM). The final
                    # attention keeps its banks alive instead — the tail
                    # recip reads the den rows straight from PSUM.
                    if not keep_zps:
                        for h2 in range(2):
                            nc.vector.tensor_copy(
                                den_j[32 * p : 32 * p + 1, h2 * SB : (h2 + 1) * SB],
                                zps[h2][64:65, :],
                            )
                    for h2 in range(2):
                        rows = slice(h2 * 64, h2 * 64 + 64)
                        nc.vector.tensor_copy(zt[p][rows, qsl], zps[h2][0:64, :])

                # --- pipelined emission (j-major) -------------------------
                # upfront filler queue: v for superblocks 0-1, all qk for
                # j=0/1 (drain_until forces completion where needed)
                for kb in range(4):
                    enq(("v", kb), gen_v(kb))
                for p in range(NP):
                    enq(("qk", p, 0), gen_qk(p, 0))
                for kb in range(4, 8):
                    enq(("v", kb), gen_v(kb))
                for p in range(NP):
                    enq(("qk", p, 1), gen_qk(p, 1))

                for j in range(NSB):
                    for p in range(NP):
                        drain_until(("qk", p, j))
                        attention(p, j, keep_zps=(j == 3 and p == 3))
                        # staggered future-work enqueues
                        if j == 1:
                            if p == 0:
                                enq(("qk", 0, 2), gen_qk(0, 2))
                                enq(("qk", 1, 2), gen_qk(1, 2))
                            elif p == 1:
                                enq(("qk", 2, 2), gen_qk(2, 2))
                                enq(("qk", 3, 2), gen_qk(3, 2))
                            elif p == 2:
                                enq(("v", 8), gen_v(8))
                                enq(("v", 9), gen_v(9))
                            else:
                                enq(("v", 10), gen_v(10))
                                enq(("v", 11), gen_v(11))
                        elif j == 2:
                            if p == 0:
                                enq(("qk", 0, 3), gen_qk(0, 3))
                                enq(("qk", 1, 3), gen_qk(1, 3))
                            elif p == 1:
                                enq(("qk", 2, 3), gen_qk(2, 3))
                                enq(("qk", 3, 3), gen_qk(3, 3))
                            elif p == 2:
                                enq(("v", 12), gen_v(12))
                                enq(("v", 13), gen_v(13))
                            else:
                                enq(("v", 14), gen_v(14))
                                enq(("v", 15), gen_v(15))
                        if j == 3 and p == 1:
                            # out(2) drips under att(2,3)/att(3,3), where
                            # the filler queue otherwise runs dry
                            enq(("out", 2), gen_out(2))
                        if p == 2:
                            # pairs 0-2 done for this superblock: their
                            # recips + chains hide under att(3, j)
                            recip_j(j, 0, 65)
                            for c in range(3):
                                enq(("ch", c, j), gen_chain(c, j))
                            if j == 3:
                                enq(("outA",), gen_outA(3))
                    if j < 3:
                        # superblock boundary: only pair 3's reciprocal and
                        # chain remain, then out-proj as next-j filler.
                        # A few dummy matmuls bridge the recip->chain serial
                        # stretch so HAM stays warm.
                        recip_j(j, 64, 97)
                        dcs = zpool.tile([128, 512], F32, tag="zp", name="dcs")
                        for _ in range(3):
                            nc.tensor.matmul(
                                dcs[96:128, 0:64],
                                warm_sb[:, 0:32],
                                warm_sb[:, 0:64],
                                start=True, stop=True,
                                tile_position=(0, 96),
                                skip_group_check=True,
                            )
                        enq(("ch", 3, j), gen_chain(3, j))
                        # defer each out-proj by one superblock: j=1 is
                        # PE-rich (out(0) there would delay the critical
                        # qk stream) while the ACT-bound j=3 starves for
                        # PE filler
                        if j >= 1:
                            enq(("out", j - 1), gen_out(j - 1))

                # tail: pair-3 norm + rank update only, pipelined by head
                # (each half's recip/bc starts as soon as its den row lands)
                drain_all()
                den_j, rcp_j, scr_j = get_denj(3)
                qsl = slice(3 * SB, 4 * SB)
                bc = fpool.tile([128, 512], F32, tag="fp", name="bc")
                for h2 in range(2):
                    hsl = slice(h2 * SB, (h2 + 1) * SB)
                    nc.scalar.activation(
                        scr_j[96:97, hsl], last_zps[h2][64:65, :],
                        mybir.ActivationFunctionType.Ln,
                    )
                    nc.scalar.activation(
                        rcp_j[96:97, hsl], scr_j[96:97, hsl],
                        mybir.ActivationFunctionType.Exp,
                        scale=-1.0,
                    )
                    nc.tensor.matmul(
                        bc[64 * h2 : 64 * h2 + 64, :],
                        ones97[96:97, 0:64],
                        rcp_j[96:97, hsl],
                        start=True, stop=True,
                        tile_position=(96, 64 * h2),
                    )
                    nc.vector.tensor_tensor(
                        zt[3][64 * h2 : 64 * h2 + 64, qsl],
                        zt[3][64 * h2 : 64 * h2 + 64, qsl],
                        bc[64 * h2 : 64 * h2 + 64, :],
                        op=mybir.AluOpType.mult,
                    )
                for _ in gen_outB(3):
                    pass

    split_multi_waits(nc)
    return nc


_CACHED = {}


def _get_nc():
    if "nc" not in _CACHED:
        _CACHED["nc"] = build_nc()
    return _CACHED["nc"]


def kernel(
    x,
    pos_embed,
    W_Q,
    b_Q,
    W_K,
    b_K,
    W_V,
    b_V,
    W_O,
    b_O,
    _want_results=False,
    _trace=False,
):
    from concourse.bass_utils import run_bass_kernel_spmd

    bf16 = ml_dtypes.bfloat16
    x = np.asarray(x, np.float32)
    W_Q = np.asarray(W_Q, np.float32)
    b_Q = np.asarray(b_Q, np.float32)
    W_K = np.asarray(W_K, np.float32)
    b_K = np.asarray(b_K, np.float32)
    W_V = np.asarray(W_V, np.float32)
    b_V = np.asarray(b_V, np.float32)
    W_O = np.asarray(W_O, np.float32)
    b_O = np.asarray(b_O, np.float32)

    in_maps = []
    for c in range(8):
        b, g = divmod(c, 2)
        hs = slice(g * H8, (g + 1) * H8)
        # [H8, M, DH] -> [M, H8*DH] with col = 64*h + d, then pre-tiled into
        # the on-chip layout: [128 part, (p, k, 128)] for Q/K (p-major),
        # [128, (k, 512)] for V, [128, (c, k, 128)] for O.
        wq_f = W_Q[hs].transpose(1, 0, 2).reshape(M, H8 * DH)
        wk_f = W_K[hs].transpose(1, 0, 2).reshape(M, H8 * DH)
        wv_f = W_V[hs].transpose(1, 0, 2).reshape(M, H8 * DH)
        wo_f = W_O[hs].reshape(H8 * DH, M)
        # [M=MK*128, NP*128] -> [MK, 128, NP, 128] -> [128, NP, MK, 128]
        wq = np.ascontiguousarray(
            wq_f.reshape(MK, 128, NP, 128).transpose(1, 2, 0, 3).reshape(128, -1)
        )
        wk = np.ascontiguousarray(
            wk_f.reshape(MK, 128, NP, 128).transpose(1, 2, 0, 3).reshape(128, -1)
        )
        wv = np.ascontiguousarray(
            wv_f.reshape(MK, 128, 512).transpose(1, 0, 2).reshape(128, -1)
        )
        # [H8*DH=NP*128, M=MK*128] -> [NP, 128, MK, 128] -> [128, NP, MK, 128]
        wo = np.ascontiguousarray(
            wo_f.reshape(NP, 128, MK, 128).transpose(1, 0, 2, 3).reshape(128, -1)
        )
        bvb = np.broadcast_to(
            b_V[hs].reshape(1, H8 * DH), (128, H8 * DH)
        )
        in_maps.append(
            {
                "x_t": np.ascontiguousarray(x[b].T).astype(bf16),
                "w_q": wq.astype(bf16),
                "w_k": wk.astype(bf16),
                "w_v": wv.astype(bf16),
                "w_o": wo.astype(bf16),
                "b_q": np.ascontiguousarray(b_Q[hs].reshape(NP, 128)),
                "b_k": np.ascontiguousarray(b_K[hs].reshape(NP, 128)),
                "b_v": np.ascontiguousarray(bvb).astype(bf16),
            }
        )

    nc = _get_nc()
    res = run_bass_kernel_spmd(nc, in_maps, list(range(8)), trace=_trace)

    out = np.empty((B, S, M), np.float32)
    for b in range(B):
        p0 = res.results[2 * b]["out_t"].astype(np.float32)
        p1 = res.results[2 * b + 1]["out_t"].astype(np.float32)
        out[b] = (p0 + p1).T + b_O
    if _want_results:
        return out, res
    return out
